# revision 23
# baseline (speedup 1.0000x reference)
"""Sigmoid-attention Bass kernel for TRN2, 8 NeuronCores (batch-parallel).

Problem (per batch element b, one per core):
    S = Q^T K            [2048, 2048]   (contract over d=128)
    P = sigmoid(S/sqrt(128))
    O = V P              [128, 2048]

Key idea vs the plain-ACT version: the sigmoid stream (4.2M elems/core)
is the bottleneck on ScalarE alone (0.833ns/elem/lane -> 33.7us). Split
it across two engines per S-tile [128, 1024]:
  - ScalarE: exact sigmoid on columns [0:512)   (~720ns/tile)
  - VectorE: fused custom-DVE op on cols [512:1024) (~660ns/tile):
        p(x) = clamp(x*(a0 + a1 x^2 + a2 x^4), -0.5, +0.5)
    which approximates sigmoid(x/sqrt(128)) - 0.5 (scale folded into the
    coefficients; deg-5 odd LS fit on the data distribution, leading
    coef > 0 so the tail overshoots and the clamp saturates exactly at
    +-0.5; end-to-end rel err ~4e-3, gate is 2e-2).
  The missing +0.5 on DVE columns is a rank-1 term 0.5*rowsum(V) added
  at drain time; rowsum(V) comes from 4x512-wide DVE reduces
  interleaved mid-stream (first needed at the h0 drain, ~15us in).
With both sigmoid engines under the PE's ~854ns/tile, the kernel is
tensor-engine bound (~65.5k cols fp32r @ 1 col/cycle @ 2.4GHz).

Layout/structure:
  - d=128 on the SBUF partition dim; S-tiles leave the PE as [n, m].
  - V^T is prepared host-side (pure layout change, like the batch
    sharding itself) and DMA'd straight into SBUF as f32r: no on-device
    transposes, no PSUM staging (GPSIMD cannot read PSUM anyway).
  - Q/K land in SBUF as f32r straight from DMA; fp32r matmuls run
    1 col/cycle with fp32 PSUM accumulation.
  - PSUM: S double-buffer 4 banks + two O-accumulator slots 4 banks.
  - O-matmuls deferred two iterations so the in-order PE stream never
    waits on the current tile's sigmoid.
  - Drains (PSUM->SBUF, only ACT/DVE can read PSUM): exact-sigmoid
    chunk = ScalarE copy, poly chunk = DVE tensor_scalar(+0.5*rowsumV);
    h1's drains run after the last sigmoid/poly, when both are free.
  - DMA on the two HWDGE queues (SP + ACT); ACT-queue issues sit before
    the first sigmoid / after the last one, so they never stall the
    activation pipeline. Input loads ordered by first use.
"""

import numpy as np

import concourse.bass as bass
import concourse.tile as tile
from concourse import bacc, mybir
from concourse.bass_utils import run_bass_kernel_spmd

B, D, N = 8, 128, 2048
NT = N // 128            # 16 n-tiles of 128
MH = 2                   # m halves
MW = N // MH             # 1024 columns per half
HW = MW // 2             # 512: ScalarE/VectorE column split within a tile
SCALE = float(1.0 / np.sqrt(128.0))
F32 = mybir.dt.float32
F32R = mybir.dt.float32r
BF16 = mybir.dt.bfloat16
SIG = mybir.ActivationFunctionType.Sigmoid
ADD = mybir.AluOpType.add

# sigmoid(x/sqrt(128)) - 0.5 ~= clamp(x*(A0 + A1 x^2 + A2 x^4), +-0.5)
# (deg-5 odd weighted-LS fit on [-4, 4] in scaled units, constrained so the
# poly stays >= 0.505 out to |x_scaled| = 32; scale folded into coefs)
PA0 = 0.2453283 * SCALE
PA1 = -0.01505521 * SCALE**3
PA2 = 0.00047899 * SCALE**5

_CACHED_NC = None


def _register_sigpoly():
    """Register the fused sigmoid-poly custom-DVE op (idempotent)."""
    from concourse import dve_ops
    from concourse.dve_spec import (
        Spec, Src0, C0, C1, C2, C3, sq, maxx, minn, lower, Zero,
        _spill_c3_to_src1, _has_src1,
    )
    from concourse.dve_uop import DveOpSpec

    name = "SIGPOLY_ANT"
    if name in dve_ops._SUB_OPCODE_FOR_NAME:
        return next(op for op in dve_ops.OPS if op.name == name)

    u = sq(Src0)
    r = ((C2 * u + C1) * u + C0) * Src0
    # clamp to [-C3, +C3]; C3 (= +0.5) delivered via in1, latched at elem 0
    body = _spill_c3_to_src1(minn(maxx(r, Zero - C3), C3))

    def ref(in0, in1, s0, s1, imm2):
        u = in0 * in0
        r = ((imm2 * u + s1) * u + s0) * in0
        return np.minimum(np.maximum(r, -in1), in1)

    spec = Spec(body=body, reference=ref)
    row = max(dve_ops._SUB_OPCODE_FOR_NAME.values()) + 1
    assert row < 0x20
    dve_ops._SUB_OPCODE_FOR_NAME[name] = row
    shas = {}
    for ver in ("v3", "v4"):
        s = DveOpSpec(name=name, opcode=row, uops=lower(spec, ver=ver),
                      rd1_en=_has_src1(spec))
        shas[ver] = s.sha(ver)
    op = dve_ops.DveOp(name, spec, False, uops_sha=shas)
    dve_ops.OPS.append(op)
    dve_ops.CUSTOM_DVE_SPECS[name] = spec
    return op


def build_nc():
    SIGPOLY = _register_sigpoly()

    nc = bacc.Bacc("TRN2", target_bir_lowering=False, debug=False, num_devices=B,
                   enable_asserts=False)
    q_ext = nc.dram_tensor("Q", [D, N], BF16, kind="ExternalInput").ap()
    k_ext = nc.dram_tensor("K", [D, N], BF16, kind="ExternalInput").ap()
    v_ext = nc.dram_tensor("V", [D, N], F32, kind="ExternalInput").ap()
    # VT is host-prepared in SBUF layout: VT[p, 128j+i] = V[i, 128j+p]
    vt_ext = nc.dram_tensor("VT", [D, N], BF16, kind="ExternalInput").ap()
    out_ext = nc.dram_tensor("out", [D, N], F32, kind="ExternalOutput").ap()

    with tile.TileContext(nc) as tc:
        with (
            tc.tile_pool(name="sb", bufs=1) as sb,
            tc.tile_pool(name="pp", bufs=4) as pp,
            tc.tile_pool(name="ob", bufs=4) as ob,
            tc.tile_pool(name="ps", bufs=2, space="PSUM") as ps,
            tc.tile_pool(name="po", bufs=2, space="PSUM") as po,
        ):
            # Q/VT live in 512-col tiles, K in per-half tiles: the matmul
            # weights (lhsT) path tracks dependencies at TILE granularity,
            # so a slice of one big DMA-written tile would wait for every
            # DMA into it. Small tiles make the first S-matmul runnable as
            # soon as its own 512-col load lands.
            qt = [sb.tile([D, 512], BF16, tag=f"q{i}", name=f"q{i}")
                  for i in range(4)]
            kh = [sb.tile([D, MW], BF16, tag=f"k{i}", name=f"k{i}")
                  for i in range(2)]
            v_sb = sb.tile([D, N], F32, tag="v", name="v_sb")
            # vtt[j][:, 128a+i] = V[i, 512j + 128a + p] (n on partitions)
            vtt = [sb.tile([D, 512], BF16, tag=f"vt{i}", name=f"vt{i}")
                   for i in range(4)]
            halft = sb.tile([D, 1], F32, tag="half", name="halft")
            rsv = sb.tile([D, 1], F32, tag="rsv", name="rsv")
            rp = [sb.tile([D, 1], F32, tag=f"rp{i}", name=f"rp{i}")
                  for i in range(4)]

            # --- prologue ------------------------------------------------
            # SP HWDGE queue, ordered by first consumption: q tiles 0-3 and
            # k h0 feed the first S-matmuls.
            nc.sync.dma_start(out=qt[0][:], in_=q_ext[:, 0:512])
            nc.sync.dma_start(out=kh[0][:], in_=k_ext[:, 0:MW])

            # ACT: sigmoid table load (~1.3us) first so it overlaps DMA,
            # then the late-needed bulk loads on the ACT HWDGE queue
            # (issued before the first sigmoid reaches the engine).
            # Junk matmuls (f32r, 1 col/cycle): start the HAM activity
            # window during the DMA wait so the clock ramps sooner; three
            # of them bridge the gap until kh0/qt0 land so the activity
            # window sees no micro-idle.
            junk = sb.tile([D, 512], F32, tag="junk", name="junk")
            nc.gpsimd.memset(junk[:], 0.0)
            junk_r = junk[:].bitcast(F32R)
            wps = po.tile([D, MW], F32, tag="o", name="warm_ps")
            for w in range(3):
                nc.tensor.matmul(wps[:, 0:512], lhsT=junk_r[:, 0:D],
                                 rhs=junk_r, start=True, stop=True)

            warm = sb.tile([D, 1], F32, tag="warm", name="warm")
            nc.gpsimd.memset(warm[:], 0.0)
            warm2 = sb.tile([D, 1], F32, tag="warm2", name="warm2")
            nc.scalar.activation(warm2[:], warm[:], SIG)
            nc.scalar.dma_start(out=kh[1][:], in_=k_ext[:, MW:N])

            nc.gpsimd.memset(halft[:], 0.5)

            # SP queue: remaining inputs in consumption order. vt cols
            # [128j, 128j+128) feed O-matmul j from iteration j+2 on.
            nc.sync.dma_start(out=vtt[0][:], in_=vt_ext[:, 0:512])
            nc.sync.dma_start(out=qt[1][:], in_=q_ext[:, 512:MW])
            nc.sync.dma_start(out=vtt[1][:], in_=vt_ext[:, 512:MW])
            nc.sync.dma_start(out=qt[2][:], in_=q_ext[:, MW:MW + 512])
            nc.sync.dma_start(out=vtt[2][:], in_=vt_ext[:, MW:MW + 512])
            nc.sync.dma_start(out=v_sb[:, 0:MW], in_=v_ext[:, 0:MW])
            nc.sync.dma_start(out=qt[3][:], in_=q_ext[:, MW + 512:N])
            nc.sync.dma_start(out=vtt[3][:], in_=vt_ext[:, MW + 512:N])
            nc.sync.dma_start(out=v_sb[:, MW:N], in_=v_ext[:, MW:N])

            # --- steady state --------------------------------------------
            # O-matmuls run two iterations behind the S-matmuls.
            pending = []

            def flush(p):
                o_ps, p01, n, h, last = p
                for c in range(2):
                    nc.tensor.matmul(
                        o_ps[:, bass.ts(c, HW)],
                        lhsT=vtt[n // 4][:, bass.ts(n % 4, D)],
                        rhs=p01[c][:],
                        start=(n == 0),
                        stop=(n == NT - 1),
                    )
                if last:
                    # Drain both chunks after both final O-matmuls, so the
                    # ScalarE copy (c0) and DVE add (c1) run in parallel.
                    # c0 = exact-sigmoid cols: ScalarE copy.
                    # c1 = poly cols (sigma-0.5): DVE add 0.5*rowsum(V).
                    final = h == MH - 1
                    for c in range(2):
                        o_out = ob.tile([D, HW], F32, tag="o_out",
                                        name=f"o_out{h}_{c}")
                        if c == 0:
                            nc.scalar.copy(o_out[:], o_ps[:, 0:HW])
                        else:
                            nc.vector.tensor_scalar(
                                out=o_out[:], in0=o_ps[:, HW:MW],
                                scalar1=rsv[:, 0:1], scalar2=None, op0=ADD)
                        dma_eng = nc.scalar if (final and c == 0) else nc.sync
                        dma_eng.dma_start(
                            out=out_ext[:, h * MW + c * HW : h * MW + (c + 1) * HW],
                            in_=o_out[:],
                        )

            for h in range(MH):
                o_ps = po.tile([D, MW], F32, tag="o", name=f"o_ps{h}")
                for n in range(NT):
                    # Per-chunk S tiles/P tiles (separate tags) keep the
                    # ACT and DVE dependency chains fully decoupled.
                    s0 = ps.tile([D, HW], F32, tag="s0", name=f"s0_{h}_{n}")
                    s1 = ps.tile([D, HW], F32, tag="s1", name=f"s1_{h}_{n}")
                    for c, sc in enumerate((s0, s1)):
                        nc.tensor.matmul(
                            sc[:],
                            lhsT=qt[n // 4][:, bass.ts(n % 4, D)],
                            rhs=kh[h][:, bass.ts(c, HW)],
                            start=True,
                            stop=True,
                        )
                    p0 = pp.tile([D, HW], BF16, tag="p0", name=f"p0_{h}_{n}")
                    p1 = pp.tile([D, HW], BF16, tag="p1", name=f"p1_{h}_{n}")
                    # exact sigmoid on [0:512) (ScalarE) ...
                    nc.scalar.activation(p0[:], s0[:], SIG, scale=SCALE)
                    # ... poly sigma-0.5 on [512:1024) (VectorE)
                    nc.vector._custom_dve(
                        SIGPOLY, out=p1[:], in0=s1[:],
                        in1=halft[:, 0:1], s0=PA0, s1=PA1, imm2=PA2)
                    # rowsum(V) pieces, interleaved where DVE has slack;
                    # only needed by the first drain (~iteration 18).
                    if h == 0 and 10 <= n <= 13:
                        nc.vector.tensor_reduce(
                            rp[n - 10][:], v_sb[:, bass.ts(n - 10, 512)],
                            mybir.AxisListType.X, ADD)
                    if h == 0 and n == 14:
                        nc.vector.tensor_tensor(rp[0][:], rp[0][:], rp[1][:], ADD)
                        nc.vector.tensor_tensor(rp[2][:], rp[2][:], rp[3][:], ADD)
                    if h == 0 and n == 15:
                        nc.vector.tensor_tensor(rp[0][:], rp[0][:], rp[2][:], ADD)
                        nc.vector.tensor_scalar(
                            out=rsv[:], in0=rp[0][:], scalar1=0.5,
                            scalar2=None, op0=mybir.AluOpType.mult)
                    pending.append((o_ps, (p0, p1), n, h, n == NT - 1))
                    if len(pending) > 2:
                        flush(pending.pop(0))
            while pending:
                flush(pending.pop(0))

    nc.compile()
    return nc


def kernel(**inputs):
    global _CACHED_NC
    import ml_dtypes
    bf16 = ml_dtypes.bfloat16
    Q = np.ascontiguousarray(inputs["Q"], dtype=np.float32).astype(bf16)
    K = np.ascontiguousarray(inputs["K"], dtype=np.float32).astype(bf16)
    V = np.ascontiguousarray(inputs["V"], dtype=np.float32)
    assert Q.shape == (B, D, N), Q.shape
    # Block-transposed V in SBUF layout: VT[b, p, 128j+i] = V[b, i, 128j+p]
    VT = np.ascontiguousarray(
        V.reshape(B, D, NT, D).transpose(0, 3, 2, 1)).reshape(B, D, N).astype(bf16)

    if _CACHED_NC is None:
        _CACHED_NC = build_nc()
    nc = _CACHED_NC

    in_maps = [{"Q": Q[i], "K": K[i], "V": V[i], "VT": VT[i]} for i in range(B)]
    res = run_bass_kernel_spmd(nc, in_maps, core_ids=list(range(B)))
    out = np.stack([res.results[i]["out"] for i in range(B)], axis=0)
    return out.astype(np.float32, copy=False)


if __name__ == "__main__":
    rng = np.random.default_rng(0)
    ins = {
        "Q": rng.standard_normal((B, D, N)).astype(np.float32),
        "K": rng.standard_normal((B, D, N)).astype(np.float32),
        "V": rng.standard_normal((B, D, N)).astype(np.float32),
    }
    out = kernel(**ins)
    print("kernel output", out.shape, out.dtype)


# revision 24
# speedup vs baseline: 1.1408x; 1.1408x over previous
"""Sigmoid-attention Bass kernel for TRN2, 8 NeuronCores (batch-parallel).

Problem (per batch element b, one per core):
    S = Q^T K            [2048, 2048]   (contract over d=128)
    P = sigmoid(S/sqrt(128))
    O = V P              [128, 2048]

Key idea vs the plain-ACT version: the sigmoid stream (4.2M elems/core)
is the bottleneck on ScalarE alone (0.833ns/elem/lane -> 33.7us). Split
it across two engines per S-tile [128, 1024]:
  - ScalarE: exact sigmoid on columns [0:512)   (~720ns/tile)
  - VectorE: fused custom-DVE op on cols [512:1024) (~660ns/tile):
        p(x) = clamp(x*(a0 + a1 x^2 + a2 x^4), -0.5, +0.5)
    which approximates sigmoid(x/sqrt(128)) - 0.5 (scale folded into the
    coefficients; deg-5 odd LS fit on the data distribution, leading
    coef > 0 so the tail overshoots and the clamp saturates exactly at
    +-0.5; end-to-end rel err ~4e-3, gate is 2e-2).
  The missing +0.5 on DVE columns is a rank-1 term 0.5*rowsum(V) added
  at drain time; rowsum(V) comes from 4x512-wide DVE reduces
  interleaved mid-stream (first needed at the h0 drain, ~15us in).
With both sigmoid engines under the PE's ~854ns/tile, the kernel is
tensor-engine bound (~65.5k cols fp32r @ 1 col/cycle @ 2.4GHz).

Layout/structure:
  - d=128 on the SBUF partition dim; S-tiles leave the PE as [n, m].
  - Q/K/V^T are cast to bf16 host-side (input DMA is chip-bandwidth
    bound with 8 cores loading at once; bf16 matmuls stream 1 col/cycle
    like f32r, measured 216ns per 512-col MM). V^T is prepared
    host-side in SBUF block layout (pure layout change, like the batch
    sharding itself): no on-device transposes, no PSUM staging (GPSIMD
    cannot read PSUM anyway).
  - Q/VT live in 512-col SBUF tiles, K per-half: matmul lhsT tracks
    dependencies at tile granularity, so big DMA-written tiles would
    stall the first S-matmul until the last load. Tiles are DMA'd in
    consumption order over both HWDGE queues (SP + ACT; ACT issues sit
    before the first sigmoid so they never stall the activations).
  - Per-chunk S/P tiles (separate pool tags) keep the ACT and DVE
    dependency chains decoupled; measured back-to-back MM spacing then
    hits the 1 col/cycle floor (+11ns) with no semaphore stalls.
  - PSUM: 2 tags x 2 bufs S chunks (4 banks) + two O-slots (4 banks).
  - O-matmuls deferred two iterations so the in-order PE stream never
    waits on the current tile's sigmoid.
  - Drains (PSUM->SBUF; only ACT/DVE can read PSUM): exact-sigmoid
    chunk = ScalarE copy, poly chunk = DVE tensor_scalar(+0.5*rowsumV),
    emitted after both final O-matmuls so they run in parallel.
  - f32r junk matmuls at t=0 start the HAM activity window during the
    first DMA wait so the 1.2->2.4GHz ramp completes sooner.
"""

import numpy as np

import concourse.bass as bass
import concourse.tile as tile
from concourse import bacc, mybir
from concourse.bass_utils import run_bass_kernel_spmd

B, D, N = 8, 128, 2048
NT = N // 128            # 16 n-tiles of 128
MH = 2                   # m halves
MW = N // MH             # 1024 columns per half
HW = MW // 2             # 512: ScalarE/VectorE column split within a tile
SCALE = float(1.0 / np.sqrt(128.0))
F32 = mybir.dt.float32
F32R = mybir.dt.float32r
BF16 = mybir.dt.bfloat16
SIG = mybir.ActivationFunctionType.Sigmoid
ADD = mybir.AluOpType.add

# sigmoid(x/sqrt(128)) - 0.5 ~= clamp(x*(A0 + A1 x^2 + A2 x^4), +-0.5)
# (deg-5 odd weighted-LS fit on [-4, 4] in scaled units, constrained so the
# poly stays >= 0.505 out to |x_scaled| = 32; scale folded into coefs)
PA0 = 0.2453283 * SCALE
PA1 = -0.01505521 * SCALE**3
PA2 = 0.00047899 * SCALE**5

_CACHED_NC = None


def _register_sigpoly():
    """Register the fused sigmoid-poly custom-DVE op (idempotent)."""
    from concourse import dve_ops
    from concourse.dve_spec import (
        Spec, Src0, C0, C1, C2, C3, sq, maxx, minn, lower, Zero,
        _spill_c3_to_src1, _has_src1,
    )
    from concourse.dve_uop import DveOpSpec

    name = "SIGPOLY_ANT"
    if name in dve_ops._SUB_OPCODE_FOR_NAME:
        return next(op for op in dve_ops.OPS if op.name == name)

    u = sq(Src0)
    r = ((C2 * u + C1) * u + C0) * Src0
    # clamp to [-C3, +C3]; C3 (= +0.5) delivered via in1, latched at elem 0
    body = _spill_c3_to_src1(minn(maxx(r, Zero - C3), C3))

    def ref(in0, in1, s0, s1, imm2):
        u = in0 * in0
        r = ((imm2 * u + s1) * u + s0) * in0
        return np.minimum(np.maximum(r, -in1), in1)

    spec = Spec(body=body, reference=ref)
    row = max(dve_ops._SUB_OPCODE_FOR_NAME.values()) + 1
    assert row < 0x20
    dve_ops._SUB_OPCODE_FOR_NAME[name] = row
    shas = {}
    for ver in ("v3", "v4"):
        s = DveOpSpec(name=name, opcode=row, uops=lower(spec, ver=ver),
                      rd1_en=_has_src1(spec))
        shas[ver] = s.sha(ver)
    op = dve_ops.DveOp(name, spec, False, uops_sha=shas)
    dve_ops.OPS.append(op)
    dve_ops.CUSTOM_DVE_SPECS[name] = spec
    return op


def build_nc():
    SIGPOLY = _register_sigpoly()

    nc = bacc.Bacc("TRN2", target_bir_lowering=False, debug=False, num_devices=B,
                   enable_asserts=False)
    q_ext = nc.dram_tensor("Q", [D, N], BF16, kind="ExternalInput").ap()
    k_ext = nc.dram_tensor("K", [D, N], BF16, kind="ExternalInput").ap()
    v_ext = nc.dram_tensor("V", [D, N], F32, kind="ExternalInput").ap()
    # VT is host-prepared in SBUF layout: VT[p, 128j+i] = V[i, 128j+p]
    vt_ext = nc.dram_tensor("VT", [D, N], BF16, kind="ExternalInput").ap()
    out_ext = nc.dram_tensor("out", [D, N], F32, kind="ExternalOutput").ap()

    with tile.TileContext(nc) as tc:
        with (
            tc.tile_pool(name="sb", bufs=1) as sb,
            tc.tile_pool(name="pp", bufs=4) as pp,
            tc.tile_pool(name="ob", bufs=4) as ob,
            tc.tile_pool(name="ps", bufs=2, space="PSUM") as ps,
            tc.tile_pool(name="po", bufs=2, space="PSUM") as po,
        ):
            # Q/VT live in 512-col tiles, K in per-half tiles: the matmul
            # weights (lhsT) path tracks dependencies at TILE granularity,
            # so a slice of one big DMA-written tile would wait for every
            # DMA into it. Small tiles make the first S-matmul runnable as
            # soon as its own 512-col load lands.
            qt = [sb.tile([D, 512], BF16, tag=f"q{i}", name=f"q{i}")
                  for i in range(4)]
            kh = [sb.tile([D, MW], BF16, tag=f"k{i}", name=f"k{i}")
                  for i in range(2)]
            v_sb = sb.tile([D, N], F32, tag="v", name="v_sb")
            # vtt[j][:, 128a+i] = V[i, 512j + 128a + p] (n on partitions)
            vtt = [sb.tile([D, 512], BF16, tag=f"vt{i}", name=f"vt{i}")
                   for i in range(4)]
            halft = sb.tile([D, 1], F32, tag="half", name="halft")
            rsv = sb.tile([D, 1], F32, tag="rsv", name="rsv")
            rp = [sb.tile([D, 1], F32, tag=f"rp{i}", name=f"rp{i}")
                  for i in range(4)]

            # --- prologue ------------------------------------------------
            # SP HWDGE queue, ordered by first consumption: q tiles 0-3 and
            # k h0 feed the first S-matmuls.
            nc.sync.dma_start(out=qt[0][:], in_=q_ext[:, 0:512])
            nc.sync.dma_start(out=kh[0][:], in_=k_ext[:, 0:MW])

            # ACT: sigmoid table load (~1.3us) first so it overlaps DMA,
            # then the late-needed bulk loads on the ACT HWDGE queue
            # (issued before the first sigmoid reaches the engine).
            # Junk matmuls (f32r, 1 col/cycle): start the HAM activity
            # window during the DMA wait so the clock ramps sooner; three
            # of them bridge the gap until kh0/qt0 land so the activity
            # window sees no micro-idle.
            junk = sb.tile([D, 512], F32, tag="junk", name="junk")
            nc.gpsimd.memset(junk[:], 0.0)
            junk_r = junk[:].bitcast(F32R)
            wps = po.tile([D, MW], F32, tag="o", name="warm_ps")
            for w in range(3):
                nc.tensor.matmul(wps[:, 0:512], lhsT=junk_r[:, 0:D],
                                 rhs=junk_r, start=True, stop=True)

            warm = sb.tile([D, 1], F32, tag="warm", name="warm")
            nc.gpsimd.memset(warm[:], 0.0)
            warm2 = sb.tile([D, 1], F32, tag="warm2", name="warm2")
            nc.scalar.activation(warm2[:], warm[:], SIG)
            nc.scalar.dma_start(out=kh[1][:], in_=k_ext[:, MW:N])

            nc.gpsimd.memset(halft[:], 0.5)

            # SP queue: remaining inputs in consumption order. vt cols
            # [128j, 128j+128) feed O-matmul j from iteration j+2 on.
            nc.sync.dma_start(out=vtt[0][:], in_=vt_ext[:, 0:512])
            nc.sync.dma_start(out=qt[1][:], in_=q_ext[:, 512:MW])
            nc.sync.dma_start(out=vtt[1][:], in_=vt_ext[:, 512:MW])
            nc.sync.dma_start(out=qt[2][:], in_=q_ext[:, MW:MW + 512])
            nc.sync.dma_start(out=vtt[2][:], in_=vt_ext[:, MW:MW + 512])
            nc.sync.dma_start(out=v_sb[:, 0:MW], in_=v_ext[:, 0:MW])
            nc.sync.dma_start(out=qt[3][:], in_=q_ext[:, MW + 512:N])
            nc.sync.dma_start(out=vtt[3][:], in_=vt_ext[:, MW + 512:N])
            nc.sync.dma_start(out=v_sb[:, MW:N], in_=v_ext[:, MW:N])

            # --- steady state --------------------------------------------
            # O-matmuls run two iterations behind the S-matmuls.
            pending = []

            def flush(p):
                o_ps, p01, n, h, last = p
                for c in range(2):
                    nc.tensor.matmul(
                        o_ps[:, bass.ts(c, HW)],
                        lhsT=vtt[n // 4][:, bass.ts(n % 4, D)],
                        rhs=p01[c][:],
                        start=(n == 0),
                        stop=(n == NT - 1),
                    )
                if last:
                    # Drain both chunks after both final O-matmuls, so the
                    # ScalarE copy (c0) and DVE add (c1) run in parallel.
                    # c0 = exact-sigmoid cols: ScalarE copy.
                    # c1 = poly cols (sigma-0.5): DVE add 0.5*rowsum(V).
                    final = h == MH - 1
                    for c in range(2):
                        o_out = ob.tile([D, HW], F32, tag="o_out",
                                        name=f"o_out{h}_{c}")
                        if c == 0:
                            nc.scalar.copy(o_out[:], o_ps[:, 0:HW])
                        else:
                            nc.vector.tensor_scalar(
                                out=o_out[:], in0=o_ps[:, HW:MW],
                                scalar1=rsv[:, 0:1], scalar2=None, op0=ADD)
                        dma_eng = nc.scalar if (final and c == 0) else nc.sync
                        dma_eng.dma_start(
                            out=out_ext[:, h * MW + c * HW : h * MW + (c + 1) * HW],
                            in_=o_out[:],
                        )

            for h in range(MH):
                o_ps = po.tile([D, MW], F32, tag="o", name=f"o_ps{h}")
                for n in range(NT):
                    # Per-chunk S tiles/P tiles (separate tags) keep the
                    # ACT and DVE dependency chains fully decoupled.
                    s0 = ps.tile([D, HW], F32, tag="s0", name=f"s0_{h}_{n}")
                    s1 = ps.tile([D, HW], F32, tag="s1", name=f"s1_{h}_{n}")
                    for c, sc in enumerate((s0, s1)):
                        nc.tensor.matmul(
                            sc[:],
                            lhsT=qt[n // 4][:, bass.ts(n % 4, D)],
                            rhs=kh[h][:, bass.ts(c, HW)],
                            start=True,
                            stop=True,
                        )
                    p0 = pp.tile([D, HW], BF16, tag="p0", name=f"p0_{h}_{n}")
                    p1 = pp.tile([D, HW], BF16, tag="p1", name=f"p1_{h}_{n}")
                    # exact sigmoid on [0:512) (ScalarE) ...
                    nc.scalar.activation(p0[:], s0[:], SIG, scale=SCALE)
                    # ... poly sigma-0.5 on [512:1024) (VectorE)
                    nc.vector._custom_dve(
                        SIGPOLY, out=p1[:], in0=s1[:],
                        in1=halft[:, 0:1], s0=PA0, s1=PA1, imm2=PA2)
                    # rowsum(V) pieces, interleaved where DVE has slack;
                    # only needed by the first drain (~iteration 18).
                    if h == 0 and 10 <= n <= 13:
                        nc.vector.tensor_reduce(
                            rp[n - 10][:], v_sb[:, bass.ts(n - 10, 512)],
                            mybir.AxisListType.X, ADD)
                    if h == 0 and n == 14:
                        nc.vector.tensor_tensor(rp[0][:], rp[0][:], rp[1][:], ADD)
                        nc.vector.tensor_tensor(rp[2][:], rp[2][:], rp[3][:], ADD)
                    if h == 0 and n == 15:
                        nc.vector.tensor_tensor(rp[0][:], rp[0][:], rp[2][:], ADD)
                        nc.vector.tensor_scalar(
                            out=rsv[:], in0=rp[0][:], scalar1=0.5,
                            scalar2=None, op0=mybir.AluOpType.mult)
                    pending.append((o_ps, (p0, p1), n, h, n == NT - 1))
                    if len(pending) > 2:
                        flush(pending.pop(0))
            while pending:
                flush(pending.pop(0))

    nc.compile()
    return nc


def kernel(**inputs):
    global _CACHED_NC
    import ml_dtypes
    bf16 = ml_dtypes.bfloat16
    Q = np.ascontiguousarray(inputs["Q"], dtype=np.float32).astype(bf16)
    K = np.ascontiguousarray(inputs["K"], dtype=np.float32).astype(bf16)
    V = np.ascontiguousarray(inputs["V"], dtype=np.float32)
    assert Q.shape == (B, D, N), Q.shape
    # Block-transposed V in SBUF layout: VT[b, p, 128j+i] = V[b, i, 128j+p]
    VT = np.ascontiguousarray(
        V.reshape(B, D, NT, D).transpose(0, 3, 2, 1)).reshape(B, D, N).astype(bf16)

    if _CACHED_NC is None:
        _CACHED_NC = build_nc()
    nc = _CACHED_NC

    in_maps = [{"Q": Q[i], "K": K[i], "V": V[i], "VT": VT[i]} for i in range(B)]
    res = run_bass_kernel_spmd(nc, in_maps, core_ids=list(range(B)))
    out = np.stack([res.results[i]["out"] for i in range(B)], axis=0)
    return out.astype(np.float32, copy=False)


if __name__ == "__main__":
    rng = np.random.default_rng(0)
    ins = {
        "Q": rng.standard_normal((B, D, N)).astype(np.float32),
        "K": rng.standard_normal((B, D, N)).astype(np.float32),
        "V": rng.standard_normal((B, D, N)).astype(np.float32),
    }
    out = kernel(**ins)
    print("kernel output", out.shape, out.dtype)


# revision 26
# speedup vs baseline: 1.1690x; 1.0247x over previous
"""Sigmoid-attention Bass kernel for TRN2, 8 NeuronCores (batch-parallel).

Problem (per batch element b, one per core):
    S = Q^T K            [2048, 2048]   (contract over d=128)
    P = sigmoid(S/sqrt(128))
    O = V P              [128, 2048]

Key idea vs the plain-ACT version: the sigmoid stream (4.2M elems/core)
is the bottleneck on ScalarE alone (0.833ns/elem/lane -> 33.7us). Split
it across two engines per S-tile [128, 1024]:
  - ScalarE: exact sigmoid on columns [0:512)   (~720ns/tile)
  - VectorE: fused custom-DVE op on cols [512:1024) (~660ns/tile):
        p(x) = clamp(x*(a0 + a1 x^2 + a2 x^4), -0.5, +0.5)
    which approximates sigmoid(x/sqrt(128)) - 0.5 (scale folded into the
    coefficients; deg-5 odd LS fit on the data distribution, leading
    coef > 0 so the tail overshoots and the clamp saturates exactly at
    +-0.5; end-to-end rel err ~4e-3, gate is 2e-2).
  The missing +0.5 on DVE columns is a rank-1 term 0.5*rowsum(V) added
  at drain time; rowsum(V) comes from 4x512-wide DVE reduces
  interleaved mid-stream (first needed at the h0 drain, ~15us in).
With both sigmoid engines under the PE's ~854ns/tile, the kernel is
tensor-engine bound (~65.5k cols fp32r @ 1 col/cycle @ 2.4GHz).

Layout/structure:
  - d=128 on the SBUF partition dim; S-tiles leave the PE as [n, m].
  - Q/K/V^T are cast to bf16 host-side (input DMA is chip-bandwidth
    bound with 8 cores loading at once; bf16 matmuls stream 1 col/cycle
    like f32r, measured 216ns per 512-col MM). V^T is prepared
    host-side in SBUF block layout (pure layout change, like the batch
    sharding itself): no on-device transposes, no PSUM staging (GPSIMD
    cannot read PSUM anyway).
  - Q/VT live in 512-col SBUF tiles, K per-half: matmul lhsT tracks
    dependencies at tile granularity, so big DMA-written tiles would
    stall the first S-matmul until the last load. Tiles are DMA'd in
    consumption order over both HWDGE queues (SP + ACT; ACT issues sit
    before the first sigmoid so they never stall the activations).
  - Per-chunk S/P tiles (separate pool tags) keep the ACT and DVE
    dependency chains decoupled; measured back-to-back MM spacing then
    hits the 1 col/cycle floor (+11ns) with no semaphore stalls.
  - PSUM: 2 tags x 2 bufs S chunks (4 banks) + two O-slots (4 banks).
  - O-matmuls deferred two iterations so the in-order PE stream never
    waits on the current tile's sigmoid.
  - Drains (PSUM->SBUF; only ACT/DVE can read PSUM): exact-sigmoid
    chunk = ScalarE copy, poly chunk = DVE tensor_scalar(+0.5*rowsumV),
    emitted after both final O-matmuls so they run in parallel.
  - f32r junk matmuls at t=0 start the HAM activity window during the
    first DMA wait so the 1.2->2.4GHz ramp completes sooner.
"""

import numpy as np

import concourse.bass as bass
import concourse.tile as tile
from concourse import bacc, mybir
from concourse.bass_utils import run_bass_kernel_spmd

B, D, N = 8, 128, 2048
NT = N // 128            # 16 n-tiles of 128
MH = 2                   # m halves
MW = N // MH             # 1024 columns per half
HW = MW // 2             # 512: ScalarE/VectorE column split within a tile
SCALE = float(1.0 / np.sqrt(128.0))
F32 = mybir.dt.float32
F32R = mybir.dt.float32r
BF16 = mybir.dt.bfloat16
SIG = mybir.ActivationFunctionType.Sigmoid
ADD = mybir.AluOpType.add

# sigmoid(x/sqrt(128)) - 0.5 ~= clamp(x*(A0 + A1 x^2 + A2 x^4), +-0.5)
# (deg-5 odd weighted-LS fit on [-4, 4] in scaled units, constrained so the
# poly stays >= 0.505 out to |x_scaled| = 32; scale folded into coefs)
PA0 = 0.2453283 * SCALE
PA1 = -0.01505521 * SCALE**3
PA2 = 0.00047899 * SCALE**5

_CACHED_NC = None


def _register_sigpoly():
    """Register the fused sigmoid-poly custom-DVE op (idempotent)."""
    from concourse import dve_ops
    from concourse.dve_spec import (
        Spec, Src0, C0, C1, C2, C3, sq, maxx, minn, lower, Zero,
        _spill_c3_to_src1, _has_src1,
    )
    from concourse.dve_uop import DveOpSpec

    name = "SIGPOLY_ANT"
    if name in dve_ops._SUB_OPCODE_FOR_NAME:
        return next(op for op in dve_ops.OPS if op.name == name)

    u = sq(Src0)
    r = ((C2 * u + C1) * u + C0) * Src0
    # clamp to [-C3, +C3]; C3 (= +0.5) delivered via in1, latched at elem 0
    body = _spill_c3_to_src1(minn(maxx(r, Zero - C3), C3))

    def ref(in0, in1, s0, s1, imm2):
        u = in0 * in0
        r = ((imm2 * u + s1) * u + s0) * in0
        return np.minimum(np.maximum(r, -in1), in1)

    spec = Spec(body=body, reference=ref)
    row = max(dve_ops._SUB_OPCODE_FOR_NAME.values()) + 1
    assert row < 0x20
    dve_ops._SUB_OPCODE_FOR_NAME[name] = row
    shas = {}
    for ver in ("v3", "v4"):
        s = DveOpSpec(name=name, opcode=row, uops=lower(spec, ver=ver),
                      rd1_en=_has_src1(spec))
        shas[ver] = s.sha(ver)
    op = dve_ops.DveOp(name, spec, False, uops_sha=shas)
    dve_ops.OPS.append(op)
    dve_ops.CUSTOM_DVE_SPECS[name] = spec
    return op


def build_nc():
    SIGPOLY = _register_sigpoly()

    nc = bacc.Bacc("TRN2", target_bir_lowering=False, debug=False, num_devices=B,
                   enable_asserts=False)
    q_ext = nc.dram_tensor("Q", [D, N], BF16, kind="ExternalInput").ap()
    k_ext = nc.dram_tensor("K", [D, N], BF16, kind="ExternalInput").ap()
    v_ext = nc.dram_tensor("V", [D, N], F32, kind="ExternalInput").ap()
    # VT is host-prepared in SBUF layout: VT[p, 128j+i] = V[i, 128j+p]
    vt_ext = nc.dram_tensor("VT", [D, N], BF16, kind="ExternalInput").ap()
    out_ext = nc.dram_tensor("out", [D, N], F32, kind="ExternalOutput").ap()

    with tile.TileContext(nc) as tc:
        with (
            tc.tile_pool(name="sb", bufs=1) as sb,
            tc.tile_pool(name="pp", bufs=4) as pp,
            tc.tile_pool(name="ob", bufs=4) as ob,
            tc.tile_pool(name="ps", bufs=2, space="PSUM") as ps,
            tc.tile_pool(name="po", bufs=2, space="PSUM") as po,
        ):
            # Q/VT live in 512-col tiles, K in per-half tiles: the matmul
            # weights (lhsT) path tracks dependencies at TILE granularity,
            # so a slice of one big DMA-written tile would wait for every
            # DMA into it. Small tiles make the first S-matmul runnable as
            # soon as its own 512-col load lands.
            qt = [sb.tile([D, 512], BF16, tag=f"q{i}", name=f"q{i}")
                  for i in range(4)]
            kh = [sb.tile([D, MW], BF16, tag=f"k{i}", name=f"k{i}")
                  for i in range(2)]
            v_sb = sb.tile([D, N], F32, tag="v", name="v_sb")
            # vtt[j][:, 128a+i] = V[i, 512j + 128a + p] (n on partitions)
            vtt = [sb.tile([D, 512], BF16, tag=f"vt{i}", name=f"vt{i}")
                   for i in range(4)]
            halft = sb.tile([D, 1], F32, tag="half", name="halft")
            rsv = sb.tile([D, 1], F32, tag="rsv", name="rsv")
            rp = [sb.tile([D, 1], F32, tag=f"rp{i}", name=f"rp{i}")
                  for i in range(4)]

            # --- prologue ------------------------------------------------
            # SP HWDGE queue, ordered by first consumption: q tiles 0-3 and
            # k h0 feed the first S-matmuls.
            nc.sync.dma_start(out=qt[0][:], in_=q_ext[:, 0:512])
            nc.sync.dma_start(out=kh[0][:], in_=k_ext[:, 0:MW])

            # ACT: sigmoid table load (~1.3us) first so it overlaps DMA,
            # then the late-needed bulk loads on the ACT HWDGE queue
            # (issued before the first sigmoid reaches the engine).
            # Junk matmuls (f32r, 1 col/cycle): start the HAM activity
            # window during the DMA wait so the clock ramps sooner; three
            # of them bridge the gap until kh0/qt0 land so the activity
            # window sees no micro-idle. The memset runs on the otherwise
            # idle VectorE so the PE isn't gated on gpsimd's stream.
            junk = sb.tile([D, 512], F32, tag="junk", name="junk")
            nc.vector.memset(junk[:], 0.0)
            junk_r = junk[:].bitcast(F32R)
            wps = po.tile([D, MW], F32, tag="o", name="warm_ps")
            for w in range(3):
                nc.tensor.matmul(wps[:, 0:512], lhsT=junk_r[:, 0:D],
                                 rhs=junk_r, start=True, stop=True)

            warm = sb.tile([D, 1], F32, tag="warm", name="warm")
            nc.gpsimd.memset(warm[:], 0.0)
            warm2 = sb.tile([D, 1], F32, tag="warm2", name="warm2")
            nc.scalar.activation(warm2[:], warm[:], SIG)
            nc.scalar.dma_start(out=kh[1][:], in_=k_ext[:, MW:N])

            nc.gpsimd.memset(halft[:], 0.5)

            # SP queue: remaining inputs in consumption order. vt cols
            # [128j, 128j+128) feed O-matmul j from iteration j+2 on.
            nc.sync.dma_start(out=vtt[0][:], in_=vt_ext[:, 0:512])
            nc.sync.dma_start(out=qt[1][:], in_=q_ext[:, 512:MW])
            nc.sync.dma_start(out=vtt[1][:], in_=vt_ext[:, 512:MW])
            nc.sync.dma_start(out=qt[2][:], in_=q_ext[:, MW:MW + 512])
            nc.sync.dma_start(out=vtt[2][:], in_=vt_ext[:, MW:MW + 512])
            nc.sync.dma_start(out=v_sb[:, 0:MW], in_=v_ext[:, 0:MW])
            nc.sync.dma_start(out=qt[3][:], in_=q_ext[:, MW + 512:N])
            nc.sync.dma_start(out=vtt[3][:], in_=vt_ext[:, MW + 512:N])
            nc.sync.dma_start(out=v_sb[:, MW:N], in_=v_ext[:, MW:N])

            # --- steady state --------------------------------------------
            # O-matmuls run two iterations behind the S-matmuls.
            pending = []

            def flush(p):
                o_ps, p01, n, h, last = p
                # On the very last tile, run the c1 matmul first: its drain
                # chain (DVE add -> sync DMA) is the kernel's critical tail.
                order = (1, 0) if (last and h == MH - 1) else (0, 1)
                for c in order:
                    nc.tensor.matmul(
                        o_ps[:, bass.ts(c, HW)],
                        lhsT=vtt[n // 4][:, bass.ts(n % 4, D)],
                        rhs=p01[c][:],
                        start=(n == 0),
                        stop=(n == NT - 1),
                    )
                if last:
                    # Drain both chunks after both final O-matmuls, so the
                    # ScalarE copy (c0) and DVE add (c1) run in parallel.
                    # c0 = exact-sigmoid cols: ScalarE copy.
                    # c1 = poly cols (sigma-0.5): DVE add 0.5*rowsum(V).
                    final = h == MH - 1
                    for c in range(2):
                        o_out = ob.tile([D, HW], F32, tag="o_out",
                                        name=f"o_out{h}_{c}")
                        if c == 0:
                            nc.scalar.copy(o_out[:], o_ps[:, 0:HW])
                        else:
                            nc.vector.tensor_scalar(
                                out=o_out[:], in0=o_ps[:, HW:MW],
                                scalar1=rsv[:, 0:1], scalar2=None, op0=ADD)
                        dma_eng = nc.scalar if (final and c == 0) else nc.sync
                        dma_eng.dma_start(
                            out=out_ext[:, h * MW + c * HW : h * MW + (c + 1) * HW],
                            in_=o_out[:],
                        )

            for h in range(MH):
                o_ps = po.tile([D, MW], F32, tag="o", name=f"o_ps{h}")
                for n in range(NT):
                    # Per-chunk S tiles/P tiles (separate tags) keep the
                    # ACT and DVE dependency chains fully decoupled.
                    s0 = ps.tile([D, HW], F32, tag="s0", name=f"s0_{h}_{n}")
                    s1 = ps.tile([D, HW], F32, tag="s1", name=f"s1_{h}_{n}")
                    for c, sc in enumerate((s0, s1)):
                        nc.tensor.matmul(
                            sc[:],
                            lhsT=qt[n // 4][:, bass.ts(n % 4, D)],
                            rhs=kh[h][:, bass.ts(c, HW)],
                            start=True,
                            stop=True,
                        )
                    p0 = pp.tile([D, HW], BF16, tag="p0", name=f"p0_{h}_{n}")
                    p1 = pp.tile([D, HW], BF16, tag="p1", name=f"p1_{h}_{n}")
                    # exact sigmoid on [0:512) (ScalarE) ...
                    nc.scalar.activation(p0[:], s0[:], SIG, scale=SCALE)
                    # ... poly sigma-0.5 on [512:1024) (VectorE)
                    nc.vector._custom_dve(
                        SIGPOLY, out=p1[:], in0=s1[:],
                        in1=halft[:, 0:1], s0=PA0, s1=PA1, imm2=PA2)
                    # rowsum(V) pieces, interleaved where DVE has slack;
                    # only needed by the first drain (~iteration 18).
                    if h == 0 and 10 <= n <= 13:
                        nc.vector.tensor_reduce(
                            rp[n - 10][:], v_sb[:, bass.ts(n - 10, 512)],
                            mybir.AxisListType.X, ADD)
                    if h == 0 and n == 14:
                        nc.vector.tensor_tensor(rp[0][:], rp[0][:], rp[1][:], ADD)
                        nc.vector.tensor_tensor(rp[2][:], rp[2][:], rp[3][:], ADD)
                    if h == 0 and n == 15:
                        nc.vector.tensor_tensor(rp[0][:], rp[0][:], rp[2][:], ADD)
                        nc.vector.tensor_scalar(
                            out=rsv[:], in0=rp[0][:], scalar1=0.5,
                            scalar2=None, op0=mybir.AluOpType.mult)
                    pending.append((o_ps, (p0, p1), n, h, n == NT - 1))
                    if len(pending) > 2:
                        flush(pending.pop(0))
            while pending:
                flush(pending.pop(0))

    nc.compile()
    return nc


def kernel(**inputs):
    global _CACHED_NC
    import ml_dtypes
    bf16 = ml_dtypes.bfloat16
    Q = np.ascontiguousarray(inputs["Q"], dtype=np.float32).astype(bf16)
    K = np.ascontiguousarray(inputs["K"], dtype=np.float32).astype(bf16)
    V = np.ascontiguousarray(inputs["V"], dtype=np.float32)
    assert Q.shape == (B, D, N), Q.shape
    # Block-transposed V in SBUF layout: VT[b, p, 128j+i] = V[b, i, 128j+p]
    VT = np.ascontiguousarray(
        V.reshape(B, D, NT, D).transpose(0, 3, 2, 1)).reshape(B, D, N).astype(bf16)

    if _CACHED_NC is None:
        _CACHED_NC = build_nc()
    nc = _CACHED_NC

    in_maps = [{"Q": Q[i], "K": K[i], "V": V[i], "VT": VT[i]} for i in range(B)]
    res = run_bass_kernel_spmd(nc, in_maps, core_ids=list(range(B)))
    out = np.stack([res.results[i]["out"] for i in range(B)], axis=0)
    return out.astype(np.float32, copy=False)


if __name__ == "__main__":
    rng = np.random.default_rng(0)
    ins = {
        "Q": rng.standard_normal((B, D, N)).astype(np.float32),
        "K": rng.standard_normal((B, D, N)).astype(np.float32),
        "V": rng.standard_normal((B, D, N)).astype(np.float32),
    }
    out = kernel(**ins)
    print("kernel output", out.shape, out.dtype)
